# revision 21
# baseline (speedup 1.0000x reference)
"""Trainium2 Bass kernel for packed-sequence attention (nn_Attention).

Sharding (8 cores): core c handles sequence c//2 and head-group c%2
(8 of 16 heads).  Each core runs an identical SPMD program:

  P0) V projection for all 8 local heads (x chunks stationary) ->
      vv [t, head*d] bf16, with ts-major x DMA so the PE starts early.
  FUSED) per head-pair p: Q/K projection of pair p interleaved with
      the attention of pair p-1 (2 proj units : 1 attention wave).
      Q/K never leave SBUF (no DRAM spill), and the softmax exp work
      on the ACT engine spreads over the whole fused window instead
      of being crammed into a separate attention phase.
      RMSNorm uses rsqrt = exp(-0.5*ln(ms+eps)) so the only ACT
      functions in the program are {Exp, Ln, Copy} -- one activation
      table, zero 1.3us table reloads.
  P3) wo: o chunks stationary, accumulated over heads in PSUM.
      o for heads 0..5 spills to DRAM during the fused phase and is
      reloaded (with wo) into the space x vacates; heads 6/7 are
      written directly into the resident tiles.

Host: pairs of cores holding the same sequence have complementary head
groups; their partial outputs are summed (row-parallel TP unshard).
"""

import json
import math
import numpy as np
import ml_dtypes
from contextlib import ExitStack

P = 128
HD = 128
BF = ml_dtypes.bfloat16


def _dedup_ldweights(nc, mybir):
    """Remove InstLdweights that reload the identical stationary operand."""
    def sig(inst):
        j = json.loads(mybir.instruction_to_pretty_json_string(inst))
        j.pop("name", None)
        j.pop("sync_info", None)
        return json.dumps(j, sort_keys=True)

    removed = 0
    for f in nc.m.functions:
        for bb in f.blocks:
            last = None
            to_del = []
            for i, inst in enumerate(bb.instructions):
                if getattr(inst, "engine", None) != mybir.EngineType.PE:
                    continue
                if isinstance(inst, mybir.InstLdweights):
                    s = sig(inst)
                    si = inst.sync_info
                    empty = si is None or (
                        len(si.on_wait) == 0 and len(si.on_update) == 0)
                    if empty and s == last:
                        to_del.append(i)
                        removed += 1
                    else:
                        last = s
                elif isinstance(inst, (mybir.InstMatmult,
                                       mybir.InstEventSemaphore)):
                    pass
                else:
                    last = None
            for i in reversed(to_del):
                del bb.instructions[i]
    return removed


def _build_program(L, C, NP, DOUT, n_cores):
    """Build the SPMD per-core program.

    L: tokens per core (sequence length), C: model/contraction dim,
    NP: local head pairs (local heads = 2*NP), DOUT: wo output dim.
    """
    import concourse.bass as bass
    import concourse.mybir as mybir
    import concourse.tile as tile
    from concourse import bacc

    dt = mybir.dt
    AF = mybir.ActivationFunctionType
    OP = mybir.AluOpType

    NHL = 2 * NP           # local heads (8)
    KC = L // P            # key chunks / token sub-tiles (16)
    CCH = C // P           # contraction chunks (16)
    NW = L // 512          # 512-wide query waves per head (4)
    JP = DOUT // 512       # wo output column panels (4)
    NU = 16                # proj units per pair: 4 half-waves x 4 utypes
    scale = 1.0 / math.sqrt(HD)
    EXPB = -math.log(16.0)  # exp(s*scale - ln16): keeps es in [~0, 25]
    EPS = 1e-5

    nc = bacc.Bacc("TRN2", target_bir_lowering=False, debug=False,
                   num_devices=n_cores)

    xT_d = nc.dram_tensor("xT", [C, L], dt.bfloat16, kind="ExternalInput").ap()
    wqkT_d = nc.dram_tensor("wqkT", [C, NP * 512], dt.bfloat16,
                            kind="ExternalInput").ap()
    wvT_d = nc.dram_tensor("wvT", [C, NHL * HD], dt.bfloat16,
                           kind="ExternalInput").ap()
    woT_d = nc.dram_tensor("woT", [NHL * HD, DOUT], dt.bfloat16,
                           kind="ExternalInput").ap()
    cosT_d = nc.dram_tensor("cosT", [P, L], dt.bfloat16, kind="ExternalInput").ap()
    sinT_d = nc.dram_tensor("sinT", [P, L], dt.bfloat16, kind="ExternalInput").ap()
    qnw_d = nc.dram_tensor("qnw", [P, 1], dt.float32, kind="ExternalInput").ap()
    knw_d = nc.dram_tensor("knw", [P, 1], dt.float32, kind="ExternalInput").ap()
    out_d = nc.dram_tensor("out", [L, DOUT], dt.float32, kind="ExternalOutput").ap()

    with tile.TileContext(nc) as tc:
        with ExitStack() as ctx:
            const = ctx.enter_context(tc.tile_pool(name="const", bufs=1))
            vvp = ctx.enter_context(tc.tile_pool(name="vvp", bufs=1))
            cosp = ctx.enter_context(tc.tile_pool(name="cosp", bufs=1))
            qkp = ctx.enter_context(tc.tile_pool(name="qkp", bufs=2))
            esp = ctx.enter_context(tc.tile_pool(name="esp", bufs=4))
            accp = ctx.enter_context(tc.tile_pool(name="accp", bufs=2))
            dpool = ctx.enter_context(tc.tile_pool(name="dpool", bufs=4,
                                                   space="DRAM"))
            ospill = ctx.enter_context(tc.tile_pool(name="ospill", bufs=1,
                                                    space="DRAM"))

            # Pre-load the one activation table containing every ACT func
            # this program uses (Exp, Ln, Copy).  Without this the
            # table-load pass alternates between the exp-only and ln-only
            # tables: one 1283ns ACT_TABLE_LOAD per rms group.
            from concourse.hw_specs import get_activation_tables
            _tabs = list(get_activation_tables(nc.m.arch).values())
            _need = {AF.Exp, AF.Ln, AF.Copy}
            _set_id = next(i for i, fs in enumerate(_tabs) if _need <= fs)
            nc.scalar.add_instruction(mybir.InstLoadActFuncSet(
                name="act_table_preload", act_func_set_id=_set_id,
                ins=[], outs=[]))

            ones_sb = const.tile([P, 1], dt.bfloat16, tag="ones", bufs=1)
            nc.vector.memset(ones_sb[:], 1.0)
            eps_sb = const.tile([P, 1], dt.float32, tag="eps", bufs=1)
            nc.vector.memset(eps_sb[:], EPS)
            expb_sb = const.tile([P, 1], dt.float32, tag="expb", bufs=1)
            nc.vector.memset(expb_sb[:], EXPB)
            qnw_sb = const.tile([P, 1], dt.float32, tag="qnw", bufs=1)
            knw_sb = const.tile([P, 1], dt.float32, tag="knw", bufs=1)
            vv_sb = vvp.tile([P, KC, NHL * HD], dt.bfloat16, tag="vv", bufs=1)
            cos_sb = cosp.tile([P, L], dt.bfloat16, tag="cos", bufs=1)
            sin_sb = cosp.tile([P, L], dt.bfloat16, tag="sin", bufs=1)

            # DRAM spill tensors for o of heads 0..5
            o_dram = [ospill.tile([P, L], dt.bfloat16, tag=f"od{h}", bufs=1,
                                  name=f"od{h}") for h in range(NHL - 2)]

            # PSUM pools for the fused phase (closed manually before P3):
            #   pq 2x + ssq 1 + s2 2x + po 2x + prs 1 = 8 banks
            man = ExitStack()
            # right-side SBUF stack: wvp (P0 only), then the fused work pool
            sbman = ExitStack()

            # ---------- emission state ----------
            wqk_tiles = {}
            qk_tiles = {}
            proj_pend = []        # (gu, ssq_fn, group_tail | None)
            pend_rope = []
            pend_dsum = []

            def drain_ssq(upto):
                while proj_pend and proj_pend[0][0] <= upto:
                    _, fn, gt = proj_pend.pop(0)
                    fn()
                    if gt is not None:
                        gt()

            def emit_rope():
                for fn in pend_rope:
                    fn()
                del pend_rope[:]

            with ExitStack() as pxs:   # x + wqk + proj PSUM: close before P3
                xp = pxs.enter_context(tc.tile_pool(name="xp", bufs=1))
                wqp = pxs.enter_context(tc.tile_pool(name="wqp", bufs=2))
                x_sb = xp.tile([P, CCH, L], dt.bfloat16, tag="x", bufs=1)
                xs = xT_d.rearrange("(cc q) w -> q cc w", q=P)

                def load_wqk(p):
                    blks = []
                    for ut in range(4):
                        blk = wqp.tile([P, CCH, P], dt.bfloat16, tag="wqk",
                                       bufs=6, name=f"wqk{p}_{ut}")
                        wqs = wqkT_d[:, p * 512 + ut * P:
                                     p * 512 + (ut + 1) * P].rearrange(
                            "(cc q) w -> q cc w", q=P)
                        nc.scalar.dma_start(blk[:], wqs[:])
                        blks.append(blk)
                    wqk_tiles[p] = blks

                def alloc_qk(p):
                    qk_tiles[p] = [[qkp.tile([P, 512], dt.bfloat16,
                                             tag=f"u{u}_{hw}", bufs=2,
                                             name=f"qk{p}_{u}_{hw}")
                                    for hw in range(NW)] for u in range(4)]

                # =============== P0: V projection (all heads) ===========
                with ExitStack() as p0:
                    wvp = p0.enter_context(tc.tile_pool(name="wvp", bufs=1,
                                                        side="right"))
                    p0ps = p0.enter_context(tc.tile_pool(name="p0ps", bufs=2,
                                                         space="PSUM"))
                    wv_sb = wvp.tile([P, CCH, NHL * HD], dt.bfloat16, tag="wv",
                                     bufs=1)
                    wvs = wvT_d.rearrange("(cc q) w -> q cc w", q=P)
                    # wv first (needed in full for ts=0), cc-major,
                    # split across both HW DGE queues for a fast start
                    for cc in range(CCH):
                        eng = nc.sync if cc % 2 == 0 else nc.scalar
                        eng.dma_start(wv_sb[:, cc, :], wvs[:, cc, :])
                    # x ts-major so the first token tiles land fast; ts0/ts1
                    # ride the emptier sync queue
                    for ts in range(KC):
                        tsl = bass.ts(ts, P)
                        eng = nc.sync if ts < 2 else nc.scalar
                        eng.dma_start(x_sb[:, :, tsl], xs[:, :, tsl])
                    # only needed from the fused phase on
                    nc.scalar.dma_start(cos_sb[:], cosT_d[:])
                    nc.scalar.dma_start(sin_sb[:], sinT_d[:])
                    nc.scalar.dma_start(qnw_sb[:], qnw_d[:])
                    nc.scalar.dma_start(knw_sb[:], knw_d[:])
                    load_wqk(0)

                    for ts in range(KC):
                        pva = p0ps.tile([P, 512], dt.float32, tag="pva", bufs=2)
                        pvb = p0ps.tile([P, 512], dt.float32, tag="pvb", bufs=2)
                        tsl = bass.ts(ts, P)
                        for cc in range(CCH):
                            st = x_sb[:, cc, tsl]
                            nc.tensor.matmul(pva[:], st, wv_sb[:, cc, 0:512],
                                             start=(cc == 0),
                                             stop=(cc == CCH - 1))
                            nc.tensor.matmul(pvb[:], st, wv_sb[:, cc, 512:1024],
                                             start=(cc == 0),
                                             stop=(cc == CCH - 1))
                        nc.vector.tensor_copy(vv_sb[:, ts, 0:512], pva[:])
                        nc.vector.tensor_copy(vv_sb[:, ts, 512:1024], pvb[:])

                # ===== FUSED: Q/K proj + RMSNorm + RoPE + attention =====
                pjps = man.enter_context(tc.tile_pool(name="pjps", bufs=2,
                                                      space="PSUM"))
                atps = man.enter_context(tc.tile_pool(name="atps", bufs=2,
                                                      space="PSUM"))
                work = sbman.enter_context(tc.tile_pool(name="work", bufs=3,
                                                        side="right"))

                group_state = {}
                ucount = [0]

                def proj_unit(p, u):
                    """One projection unit (16 matmuls) + lagged tails.

                    u = ut*4 + hw; ut in {q0,q1,k0,k1}, hw = 512-token wave.
                    (ut-outer so each weight block is done early and its
                    ring slot frees for the next pair's prefetch.)
                    ssq matmuls lag 2 units; ln/exp batch groups of 2.
                    """
                    gu = ucount[0]
                    ucount[0] += 1
                    ut, hw = (2, 0, 3, 1)[u // 4], u % 4
                    tsl = bass.ts(hw, 512)
                    blk = wqk_tiles[p][ut]
                    pq = pjps.tile([P, 512], dt.float32, tag="pq", bufs=2)
                    for cc in range(CCH):
                        nc.tensor.matmul(pq[:], blk[:, cc, :],
                                         x_sb[:, cc, tsl],
                                         start=(cc == 0), stop=(cc == CCH - 1))
                    drain_ssq(gu - 2)
                    qraw = work.tile([P, 512], dt.bfloat16, tag="qraw", bufs=6,
                                     name=f"qraw{p}_{u}")
                    nc.vector.tensor_copy(qraw[:], pq[:])
                    q2 = work.tile([P, 512], dt.bfloat16, tag="q2", bufs=2)
                    nc.vector.tensor_mul(q2[:], qraw[:], qraw[:])

                    r = gu % 2           # row within ssq group (0->0, 1->32)
                    if r == 0:
                        group_state["ssq"] = pjps.tile([33, 512], dt.float32,
                                                       tag="ssq", bufs=1,
                                                       name=f"ssq{gu}")
                        group_state["members"] = []
                    ssq = group_state["ssq"]
                    group_state["members"].append((p, u, qraw))

                    def ssq_mm(ssq=ssq, r=r, q2=q2):
                        nc.tensor.matmul(ssq[32 * r:32 * r + 1, :],
                                         ones_sb[:], q2[:],
                                         start=True, stop=True)

                    gt = None
                    if r == 1:
                        mem = list(group_state["members"])

                        def group_tail(ssq=ssq, mem=mem):
                            # rows 1..31 are never written; the Ln/Exp of
                            # those lanes is garbage nothing reads.
                            lnv = work.tile([33, 512], dt.bfloat16, tag="lnv",
                                            bufs=2)
                            nc.scalar.activation(lnv[:], ssq[:], AF.Ln,
                                                 bias=eps_sb[0:33, :],
                                                 scale=1.0 / HD)
                            rs = work.tile([33, 512], dt.float32, tag="rs",
                                           bufs=2)
                            nc.scalar.activation(rs[:], lnv[:], AF.Exp,
                                                 scale=-0.5)
                            rsd = dpool.tile([2, 512], dt.float32, tag="rsd",
                                             bufs=2)
                            nc.sync.dma_start(rsd[0:1, :], rs[0:1, :])
                            nc.sync.dma_start(rsd[1:2, :], rs[32:33, :])

                            def rope_tail(mem=mem, rsd=rsd):
                                for i, (p_, u_, qraw_) in enumerate(mem):
                                    ut_, hw_ = (2, 0, 3, 1)[u_ // 4], u_ % 4
                                    tsl_ = bass.ts(hw_, 512)
                                    wnorm = qnw_sb if ut_ < 2 else knw_sb
                                    rsb = work.tile([P, 512], dt.float32,
                                                    tag="rsb", bufs=2)
                                    nc.sync.dma_start(
                                        rsb[:],
                                        rsd[i:i + 1, :].to_broadcast((P, 512)))
                                    qs = work.tile([P, 512], dt.bfloat16,
                                                   tag="qs", bufs=2)
                                    nc.vector.scalar_tensor_tensor(
                                        qs[:], qraw_[:], wnorm[:], rsb[:],
                                        op0=OP.mult, op1=OP.mult)
                                    qsw = work.tile([P, 512], dt.bfloat16,
                                                    tag="qsw", bufs=2)
                                    nc.sync.dma_start(qsw[0:64, :],
                                                      qs[64:128, :])
                                    nc.sync.dma_start(qsw[64:128, :],
                                                      qs[0:64, :])
                                    t1 = work.tile([P, 512], dt.bfloat16,
                                                   tag="t1", bufs=2)
                                    nc.vector.tensor_mul(t1[:], qs[:],
                                                         cos_sb[:, tsl_])
                                    t2 = work.tile([P, 512], dt.bfloat16,
                                                   tag="t2", bufs=2)
                                    nc.vector.tensor_mul(t2[:], qsw[:],
                                                         sin_sb[:, tsl_])
                                    nc.vector.tensor_add(
                                        qk_tiles[p_][ut_][hw_][:],
                                        t1[:], t2[:])
                            pend_rope.append(rope_tail)
                        gt = group_tail
                    proj_pend.append((gu, ssq_mm, gt))

                def attn_wave(p, l, w, o_dest):
                    """Attention for head h=2p+l, 512-query wave w."""
                    h = 2 * p + l
                    qr = qk_tiles[p][l][w]
                    kr = qk_tiles[p][2 + l]
                    qsl = bass.ts(w, 512)
                    hsl = slice(h * HD, (h + 1) * HD)
                    po = atps.tile([P, 512], dt.float32, tag="po", bufs=2)
                    es_tiles = []
                    acc = None

                    def emit_pv(kc):
                        nc.tensor.matmul(po[:], vv_sb[:, kc, hsl],
                                         es_tiles[kc // 2][:, kc % 2, :],
                                         start=(kc == 0), stop=(kc == KC - 1))

                    for kc in range(KC):
                        s2 = atps.tile([P, 512], dt.float32, tag="s2", bufs=2)
                        nc.tensor.matmul(s2[:],
                                         kr[kc // 4][:, (kc % 4) * P:
                                                     (kc % 4 + 1) * P],
                                         qr[:], start=True, stop=True)
                        if kc % 2 == 0:
                            es_tiles.append(esp.tile([P, 2, 512], dt.bfloat16,
                                                     tag="es", bufs=3,
                                                     name=f"es{h}_{w}_{kc}"))
                        nc.scalar.activation(es_tiles[-1][:, kc % 2, :], s2[:],
                                             AF.Exp, bias=expb_sb[:],
                                             scale=scale)
                        if kc == 4 and pend_dsum:
                            pend_dsum.pop(0)()
                        if kc % 2 == 1:
                            es = es_tiles[-1]
                            na = accp.tile([P, 2, 512], dt.bfloat16, tag="acc",
                                           bufs=2)
                            if acc is None:
                                nc.vector.tensor_copy(na[:], es[:])
                            else:
                                nc.vector.tensor_add(na[:], acc[:], es[:])
                            acc = na
                        if kc >= 2:
                            emit_pv(kc - 2)
                    emit_pv(KC - 2)
                    emit_pv(KC - 1)
                    dest, spill_fn = o_dest(h, qsl, w)

                    def dsum(po=po, acc=acc, dest=dest, spill_fn=spill_fn):
                        prs = atps.tile([1, 512], dt.float32, tag="prs",
                                        bufs=1)
                        for i in range(2):
                            nc.tensor.matmul(prs[:], ones_sb[:], acc[:, i, :],
                                             start=(i == 0), stop=(i == 1))
                        rr = work.tile([1, 512], dt.float32, tag="rr", bufs=2)
                        nc.vector.reciprocal_approx_fast(rr[:], prs[:])
                        rrd = dpool.tile([1, 512], dt.float32, tag="rrd",
                                         bufs=2)
                        nc.sync.dma_start(rrd[:], rr[:])
                        rrb = work.tile([P, 512], dt.float32, tag="rrb",
                                        bufs=2)
                        nc.sync.dma_start(rrb[:],
                                          rrd[:].to_broadcast((P, 512)))
                        nc.vector.tensor_mul(dest, po[:], rrb[:])
                        if spill_fn is not None:
                            spill_fn()
                    pend_dsum.append(dsum)

                # o destination during pairs 0..2: staging tile + DRAM spill
                def o_dest_spill(h, qsl, w):
                    ost = work.tile([P, 512], dt.bfloat16, tag="ost", bufs=2)

                    def spill(ost=ost, h=h, qsl=qsl):
                        nc.sync.dma_start(o_dram[h][:, qsl], ost[:])
                    return ost[:], spill

                # ---- fused steps ----
                alloc_qk(0)
                load_wqk(1)
                for u in range(NU):           # step 0: proj pair 0 only
                    proj_unit(0, u)
                    if u % 4 == 1:
                        emit_rope()
                drain_ssq(10 ** 9)
                emit_rope()

                for p in range(1, NP):        # steps 1..3
                    if p + 1 < NP:
                        load_wqk(p + 1)
                    alloc_qk(p)
                    for j in range(8):
                        proj_unit(p, 2 * j)
                        proj_unit(p, 2 * j + 1)
                        l, w = divmod(j, NW)
                        attn_wave(p - 1, l, w, o_dest_spill)
                        emit_rope()
                # flush remaining dsums (spills) before the rope drain so
                # their small DMAs aren't stuck behind it on the sync queue
                for fn in pend_dsum:
                    fn()
                del pend_dsum[:]
                drain_ssq(10 ** 9)
                emit_rope()

            # x/wqk freed; load wo + reload o(h0..5) into their space
            with ExitStack() as p3x:
                wop = p3x.enter_context(tc.tile_pool(name="wop", bufs=1))
                p3op = p3x.enter_context(tc.tile_pool(name="p3op", bufs=1))
                wo_sb = wop.tile([P, NHL, DOUT], dt.bfloat16, tag="wo", bufs=1)
                wos = woT_d.rearrange("(h q) j -> q h j", q=P)
                for h in range(NHL):
                    nc.scalar.dma_start(wo_sb[:, h, :], wos[:, h, :])
                o_tiles = [p3op.tile([P, L], dt.bfloat16, tag=f"o{h}", bufs=1,
                                     name=f"o{h}") for h in range(NHL)]
                for h in range(NHL - 2):
                    nc.sync.dma_start(o_tiles[h][:], o_dram[h][:])

                # final attention: pair 3, o written directly into o_tiles
                def o_dest_direct(h, qsl, w):
                    return o_tiles[h][:, qsl], None

                for j in range(8):
                    l, w = divmod(j, NW)
                    attn_wave(NP - 1, l, w, o_dest_direct)
                for fn in pend_dsum:
                    fn()
                del pend_dsum[:]
                sbman.close()  # release the fused work pool (right side)
                man.close()    # release fused-phase PSUM pools

                # ======================= P3: wo ========================
                with ExitStack() as p3:
                    w3 = p3.enter_context(tc.tile_pool(name="w3", bufs=3))
                    p3ps = p3.enter_context(tc.tile_pool(name="p3ps", bufs=2,
                                                         space="PSUM"))
                    for tt in range(KC):
                        pw = p3ps.tile([P, JP, 512], dt.float32, tag="pw",
                                       bufs=2)
                        ttsl = bass.ts(tt, P)
                        for h in range(NHL):
                            ost = o_tiles[h][:, ttsl]
                            for jp in range(JP):
                                nc.tensor.matmul(pw[:, jp, :], ost,
                                                 wo_sb[:, h, bass.ts(jp, 512)],
                                                 start=(h == 0),
                                                 stop=(h == NHL - 1))
                        for jp in range(JP):
                            osb = w3.tile([P, 512], dt.float32, tag="outsb",
                                          bufs=3)
                            nc.vector.tensor_copy(osb[:], pw[:, jp, :])
                            eng = nc.sync if jp % 2 == 0 else nc.scalar
                            eng.dma_start(out_d[ttsl, bass.ts(jp, 512)],
                                          osb[:])

    import concourse.mybir as mybir_
    n_rm = _dedup_ldweights(nc, mybir_)
    print(f"[kernel] dedup removed {n_rm} redundant ldweights")
    nc.compile()
    return nc


def _host_prepare(x, rope_cos, rope_sin, wqkv, wo, q_norm_w, k_norm_w,
                  L, C, NP, DOUT, n_cores):
    """Build per-core input dicts."""
    NH_TOT = wqkv.shape[0] // 3 // HD
    NHL = 2 * NP
    perm = np.concatenate([np.arange(0, HD, 2), np.arange(1, HD, 2)])  # deinterleave

    qn_p = np.ascontiguousarray(q_norm_w[perm].reshape(HD, 1)).astype(np.float32)
    kn_p = np.ascontiguousarray(k_norm_w[perm].reshape(HD, 1)).astype(np.float32)

    wq = wqkv[0 * NH_TOT * HD:1 * NH_TOT * HD].reshape(NH_TOT, HD, C)
    wk = wqkv[1 * NH_TOT * HD:2 * NH_TOT * HD].reshape(NH_TOT, HD, C)
    wv = wqkv[2 * NH_TOT * HD:3 * NH_TOT * HD].reshape(NH_TOT, HD, C)

    in_maps = []
    for c in range(n_cores):
        b = c // 2
        hg = c % 2
        heads = list(range(hg * NHL, hg * NHL + NHL))
        xb = x[b * L:(b + 1) * L]                       # [L, C]
        xT = np.ascontiguousarray(xb.T).astype(BF)      # [C, L]

        qk_blocks = []
        for pidx in range(NP):
            h0, h1 = heads[2 * pidx], heads[2 * pidx + 1]
            qk_blocks += [wq[h0][perm], wq[h1][perm],
                          wk[h0][perm], wk[h1][perm]]
        wqkT = np.ascontiguousarray(
            np.concatenate(qk_blocks, axis=0).T).astype(BF)   # [C, NP*512]
        wvT = np.ascontiguousarray(
            np.concatenate([wv[h] for h in heads], axis=0).T).astype(BF)

        woT_rows = wo[:, heads[0] * HD:(heads[-1] + 1) * HD].T  # [NHL*HD, DOUT]
        woT = np.ascontiguousarray(woT_rows).astype(BF)

        cosb = rope_cos[b * L:(b + 1) * L].T            # [64, L]
        sinb = rope_sin[b * L:(b + 1) * L].T
        cosT = np.ascontiguousarray(np.concatenate([cosb, cosb], 0)).astype(BF)
        sinT = np.ascontiguousarray(np.concatenate([-sinb, sinb], 0)).astype(BF)

        in_maps.append({
            "xT": xT, "wqkT": wqkT, "wvT": wvT, "woT": woT,
            "cosT": cosT, "sinT": sinT, "qnw": qn_p, "knw": kn_p,
        })
    return in_maps


def _reference_numpy(x, rope_cos, rope_sin, cu, max_length,
                     wqkv, wo, q_norm_w, k_norm_w):
    """Pure-numpy fallback (exact reference math) for non-uniform cu."""
    T, dim = x.shape
    nh = dim // HD
    qkv = (x @ wqkv.T).reshape(T, 3, nh, HD)
    q, k, v = qkv[:, 0], qkv[:, 1], qkv[:, 2]

    def rmsnorm(t, w):
        return t / np.sqrt((t * t).mean(-1, keepdims=True) + 1e-5) * w

    def rope(t):
        tr = t.reshape(t.shape[:-1] + (HD // 2, 2))
        e, o = tr[..., 0], tr[..., 1]
        cc = rope_cos[:, None, :]
        ss = rope_sin[:, None, :]
        return np.stack([e * cc - o * ss, e * ss + o * cc], -1).reshape(t.shape)

    q = rope(rmsnorm(q, q_norm_w))
    k = rope(rmsnorm(k, k_norm_w))
    o = np.zeros((T, nh, HD), np.float32)
    nb = len(cu) - 1
    for i in range(nb):
        s, e_ = int(cu[i]), int(cu[i + 1])
        if e_ <= s:
            continue
        qs_, ks_, vs_ = q[s:e_], k[s:e_], v[s:e_]
        sc = np.einsum("lhd,mhd->hlm", qs_, ks_) / math.sqrt(HD)
        sc = sc - sc.max(-1, keepdims=True)
        a = np.exp(sc)
        a /= a.sum(-1, keepdims=True)
        o[s:e_] = np.einsum("hlm,mhd->lhd", a, vs_)
    return (o.reshape(T, dim) @ wo.T).astype(np.float32)


def kernel(x, rope_cos, rope_sin, cu, max_length, wqkv, wo, q_norm_w, k_norm_w):
    x = np.asarray(x, np.float32)
    rope_cos = np.asarray(rope_cos, np.float32)
    rope_sin = np.asarray(rope_sin, np.float32)
    cu = np.asarray(cu)
    wqkv = np.asarray(wqkv, np.float32)
    wo = np.asarray(wo, np.float32)
    q_norm_w = np.asarray(q_norm_w, np.float32)
    k_norm_w = np.asarray(k_norm_w, np.float32)

    T, C = x.shape
    N_CORES = 8
    L = T // 4
    expect_cu = np.arange(5) * L
    if (len(cu) != 5 or not np.array_equal(np.asarray(cu).ravel(), expect_cu)
            or T % 4 != 0 or L % 512 != 0 or C % P != 0):
        return _reference_numpy(x, rope_cos, rope_sin, cu, max_length,
                                wqkv, wo, q_norm_w, k_norm_w)

    NP = (C // HD) // 2 // 2          # local head pairs = NH/2/2
    DOUT = wo.shape[0]

    from concourse.bass_utils import run_bass_kernel_spmd

    nc = _build_program(L, C, NP, DOUT, N_CORES)
    in_maps = _host_prepare(x, rope_cos, rope_sin, wqkv, wo, q_norm_w, k_norm_w,
                            L, C, NP, DOUT, N_CORES)
    res = run_bass_kernel_spmd(nc, in_maps, list(range(N_CORES)))

    out = np.empty((T, DOUT), np.float32)
    for b in range(4):
        out[b * L:(b + 1) * L] = (res.results[2 * b]["out"]
                                  + res.results[2 * b + 1]["out"])
    return out


# revision 22
# speedup vs baseline: 1.0192x; 1.0192x over previous
"""Trainium2 Bass kernel for packed-sequence attention (nn_Attention).

Sharding (8 cores): core c handles sequence c//2 and head-group c%2
(8 of 16 heads).  Each core runs an identical SPMD program:

  P0) V projection for all 8 local heads (x chunks stationary) ->
      vv [t, head*d] bf16, with ts-major x DMA so the PE starts early.
  FUSED) per head-pair p: Q/K projection of pair p interleaved with
      the attention of pair p-1 (2 proj units : 1 attention wave).
      Q/K never leave SBUF (no DRAM spill), and the softmax exp work
      on the ACT engine spreads over the whole fused window instead
      of being crammed into a separate attention phase.
      RMSNorm uses rsqrt = exp(-0.5*ln(ms+eps)) so the only ACT
      functions in the program are {Exp, Ln, Copy} -- one activation
      table, zero 1.3us table reloads.
  P3) wo: o chunks stationary, accumulated over heads in PSUM.
      o for heads 0..5 spills to DRAM during the fused phase and is
      reloaded (with wo) into the space x vacates; heads 6/7 are
      written directly into the resident tiles.

Host: pairs of cores holding the same sequence have complementary head
groups; their partial outputs are summed (row-parallel TP unshard).
"""

import json
import math
import numpy as np
import ml_dtypes
from contextlib import ExitStack

P = 128
HD = 128
BF = ml_dtypes.bfloat16


def _dedup_ldweights(nc, mybir):
    """Remove InstLdweights that reload the identical stationary operand."""
    def sig(inst):
        j = json.loads(mybir.instruction_to_pretty_json_string(inst))
        j.pop("name", None)
        j.pop("sync_info", None)
        return json.dumps(j, sort_keys=True)

    removed = 0
    for f in nc.m.functions:
        for bb in f.blocks:
            last = None
            to_del = []
            for i, inst in enumerate(bb.instructions):
                if getattr(inst, "engine", None) != mybir.EngineType.PE:
                    continue
                if isinstance(inst, mybir.InstLdweights):
                    s = sig(inst)
                    si = inst.sync_info
                    empty = si is None or (
                        len(si.on_wait) == 0 and len(si.on_update) == 0)
                    if empty and s == last:
                        to_del.append(i)
                        removed += 1
                    else:
                        last = s
                elif isinstance(inst, (mybir.InstMatmult,
                                       mybir.InstEventSemaphore)):
                    pass
                else:
                    last = None
            for i in reversed(to_del):
                del bb.instructions[i]
    return removed


def _build_program(L, C, NP, DOUT, n_cores):
    """Build the SPMD per-core program.

    L: tokens per core (sequence length), C: model/contraction dim,
    NP: local head pairs (local heads = 2*NP), DOUT: wo output dim.
    """
    import concourse.bass as bass
    import concourse.mybir as mybir
    import concourse.tile as tile
    from concourse import bacc

    dt = mybir.dt
    AF = mybir.ActivationFunctionType
    OP = mybir.AluOpType

    NHL = 2 * NP           # local heads (8)
    KC = L // P            # key chunks / token sub-tiles (16)
    CCH = C // P           # contraction chunks (16)
    NW = L // 512          # 512-wide query waves per head (4)
    JP = DOUT // 512       # wo output column panels (4)
    NU = 16                # proj units per pair: 4 half-waves x 4 utypes
    scale = 1.0 / math.sqrt(HD)
    EXPB = -math.log(16.0)  # exp(s*scale - ln16): keeps es in [~0, 25]
    EPS = 1e-5

    nc = bacc.Bacc("TRN2", target_bir_lowering=False, debug=False,
                   num_devices=n_cores)

    xT_d = nc.dram_tensor("xT", [C, L], dt.bfloat16, kind="ExternalInput").ap()
    wqkT_d = nc.dram_tensor("wqkT", [C, NP * 512], dt.bfloat16,
                            kind="ExternalInput").ap()
    wvT_d = nc.dram_tensor("wvT", [C, NHL * HD], dt.bfloat16,
                           kind="ExternalInput").ap()
    woT_d = nc.dram_tensor("woT", [NHL * HD, DOUT], dt.bfloat16,
                           kind="ExternalInput").ap()
    cosT_d = nc.dram_tensor("cosT", [P, L], dt.bfloat16, kind="ExternalInput").ap()
    sinT_d = nc.dram_tensor("sinT", [P, L], dt.bfloat16, kind="ExternalInput").ap()
    qnw_d = nc.dram_tensor("qnw", [P, 1], dt.float32, kind="ExternalInput").ap()
    knw_d = nc.dram_tensor("knw", [P, 1], dt.float32, kind="ExternalInput").ap()
    out_d = nc.dram_tensor("out", [L, DOUT], dt.float32, kind="ExternalOutput").ap()

    with tile.TileContext(nc) as tc:
        with ExitStack() as ctx:
            const = ctx.enter_context(tc.tile_pool(name="const", bufs=1))
            vvp = ctx.enter_context(tc.tile_pool(name="vvp", bufs=1))
            cosp = ctx.enter_context(tc.tile_pool(name="cosp", bufs=1))
            qkp = ctx.enter_context(tc.tile_pool(name="qkp", bufs=2))
            esp = ctx.enter_context(tc.tile_pool(name="esp", bufs=4))
            accp = ctx.enter_context(tc.tile_pool(name="accp", bufs=2))
            dpool = ctx.enter_context(tc.tile_pool(name="dpool", bufs=4,
                                                   space="DRAM"))
            ospill = ctx.enter_context(tc.tile_pool(name="ospill", bufs=1,
                                                    space="DRAM"))

            # Pre-load the one activation table containing every ACT func
            # this program uses (Exp, Ln, Copy).  Without this the
            # table-load pass alternates between the exp-only and ln-only
            # tables: one 1283ns ACT_TABLE_LOAD per rms group.
            from concourse.hw_specs import get_activation_tables
            _tabs = list(get_activation_tables(nc.m.arch).values())
            _need = {AF.Exp, AF.Ln, AF.Copy}
            _set_id = next(i for i, fs in enumerate(_tabs) if _need <= fs)
            nc.scalar.add_instruction(mybir.InstLoadActFuncSet(
                name="act_table_preload", act_func_set_id=_set_id,
                ins=[], outs=[]))

            ones_sb = const.tile([P, 1], dt.bfloat16, tag="ones", bufs=1)
            nc.vector.memset(ones_sb[:], 1.0)
            eps_sb = const.tile([P, 1], dt.float32, tag="eps", bufs=1)
            nc.vector.memset(eps_sb[:], EPS)
            expb_sb = const.tile([P, 1], dt.float32, tag="expb", bufs=1)
            nc.vector.memset(expb_sb[:], EXPB)
            qnw_sb = const.tile([P, 1], dt.float32, tag="qnw", bufs=1)
            knw_sb = const.tile([P, 1], dt.float32, tag="knw", bufs=1)
            vv_sb = vvp.tile([P, KC, NHL * HD], dt.bfloat16, tag="vv", bufs=1)
            cos_sb = cosp.tile([P, L], dt.bfloat16, tag="cos", bufs=1)
            sin_sb = cosp.tile([P, L], dt.bfloat16, tag="sin", bufs=1)

            # DRAM spill tensors for o of heads 0..5
            o_dram = [ospill.tile([P, L], dt.bfloat16, tag=f"od{h}", bufs=1,
                                  name=f"od{h}") for h in range(NHL - 2)]

            # PSUM pools for the fused phase (closed manually before P3):
            #   pq 2x + ssq 1 + s2 2x + po 2x + prs 1 = 8 banks
            man = ExitStack()
            # right-side SBUF stack: wvp (P0 only), then the fused work pool
            sbman = ExitStack()

            # ---------- emission state ----------
            wqk_tiles = {}
            qk_tiles = {}
            proj_pend = []        # (gu, ssq_fn, group_tail | None)
            pend_rope = []
            pend_dsum = []

            def drain_ssq(upto):
                while proj_pend and proj_pend[0][0] <= upto:
                    _, fn, gt = proj_pend.pop(0)
                    fn()
                    if gt is not None:
                        gt()

            def emit_rope():
                for fn in pend_rope:
                    fn()
                del pend_rope[:]

            with ExitStack() as pxs:   # x + wqk + proj PSUM: close before P3
                xp = pxs.enter_context(tc.tile_pool(name="xp", bufs=1))
                wqp = pxs.enter_context(tc.tile_pool(name="wqp", bufs=2))
                x_sb = xp.tile([P, CCH, L], dt.bfloat16, tag="x", bufs=1)
                xs = xT_d.rearrange("(cc q) w -> q cc w", q=P)

                def load_wqk(p):
                    blks = []
                    for ut in range(4):
                        blk = wqp.tile([P, CCH, P], dt.bfloat16, tag="wqk",
                                       bufs=6, name=f"wqk{p}_{ut}")
                        wqs = wqkT_d[:, p * 512 + ut * P:
                                     p * 512 + (ut + 1) * P].rearrange(
                            "(cc q) w -> q cc w", q=P)
                        nc.scalar.dma_start(blk[:], wqs[:])
                        blks.append(blk)
                    wqk_tiles[p] = blks

                def alloc_qk(p):
                    qk_tiles[p] = [[qkp.tile([P, 512], dt.bfloat16,
                                             tag=f"u{u}_{hw}", bufs=2,
                                             name=f"qk{p}_{u}_{hw}")
                                    for hw in range(NW)] for u in range(4)]

                # =============== P0: V projection (all heads) ===========
                with ExitStack() as p0:
                    wvp = p0.enter_context(tc.tile_pool(name="wvp", bufs=1,
                                                        side="right"))
                    p0ps = p0.enter_context(tc.tile_pool(name="p0ps", bufs=2,
                                                         space="PSUM"))
                    wv_sb = wvp.tile([P, CCH, NHL * HD], dt.bfloat16, tag="wv",
                                     bufs=1)
                    wvs = wvT_d.rearrange("(cc q) w -> q cc w", q=P)
                    # P0 runs in two output-column halves so the PE can
                    # start after just wv's first half (2MB) + x's first
                    # token block, instead of the full 4MB of wv.
                    nc.sync.dma_start(x_sb[:, :, 0:P], xs[:, :, 0:P])
                    for cc in range(CCH):
                        nc.sync.dma_start(wv_sb[:, cc, 0:512],
                                          wvs[:, cc, 0:512])
                    nc.sync.dma_start(x_sb[:, :, P:2 * P], xs[:, :, P:2 * P])
                    for cc in range(CCH):
                        nc.scalar.dma_start(wv_sb[:, cc, 512:1024],
                                            wvs[:, cc, 512:1024])
                    for ts in range(2, KC):
                        tsl = bass.ts(ts, P)
                        nc.scalar.dma_start(x_sb[:, :, tsl], xs[:, :, tsl])
                    # only needed from the fused phase on
                    nc.scalar.dma_start(cos_sb[:], cosT_d[:])
                    nc.scalar.dma_start(sin_sb[:], sinT_d[:])
                    nc.scalar.dma_start(qnw_sb[:], qnw_d[:])
                    nc.scalar.dma_start(knw_sb[:], knw_d[:])
                    load_wqk(0)

                    for half in range(2):
                        hsl0 = slice(half * 512, (half + 1) * 512)
                        for ts in range(KC):
                            pv = p0ps.tile([P, 512], dt.float32, tag="pv",
                                           bufs=2)
                            tsl = bass.ts(ts, P)
                            for cc in range(CCH):
                                nc.tensor.matmul(pv[:], x_sb[:, cc, tsl],
                                                 wv_sb[:, cc, hsl0],
                                                 start=(cc == 0),
                                                 stop=(cc == CCH - 1))
                            nc.vector.tensor_copy(vv_sb[:, ts, hsl0], pv[:])

                # ===== FUSED: Q/K proj + RMSNorm + RoPE + attention =====
                pjps = man.enter_context(tc.tile_pool(name="pjps", bufs=2,
                                                      space="PSUM"))
                atps = man.enter_context(tc.tile_pool(name="atps", bufs=2,
                                                      space="PSUM"))
                work = sbman.enter_context(tc.tile_pool(name="work", bufs=3,
                                                        side="right"))

                group_state = {}
                ucount = [0]

                def proj_unit(p, u):
                    """One projection unit (16 matmuls) + lagged tails.

                    u = ut*4 + hw; ut in {q0,q1,k0,k1}, hw = 512-token wave.
                    (ut-outer so each weight block is done early and its
                    ring slot frees for the next pair's prefetch.)
                    ssq matmuls lag 2 units; ln/exp batch groups of 2.
                    """
                    gu = ucount[0]
                    ucount[0] += 1
                    ut, hw = (2, 0, 3, 1)[u // 4], u % 4
                    tsl = bass.ts(hw, 512)
                    blk = wqk_tiles[p][ut]
                    pq = pjps.tile([P, 512], dt.float32, tag="pq", bufs=2)
                    for cc in range(CCH):
                        nc.tensor.matmul(pq[:], blk[:, cc, :],
                                         x_sb[:, cc, tsl],
                                         start=(cc == 0), stop=(cc == CCH - 1))
                    drain_ssq(gu - 2)
                    qraw = work.tile([P, 512], dt.bfloat16, tag="qraw", bufs=6,
                                     name=f"qraw{p}_{u}")
                    nc.vector.tensor_copy(qraw[:], pq[:])
                    q2 = work.tile([P, 512], dt.bfloat16, tag="q2", bufs=2)
                    nc.vector.tensor_mul(q2[:], qraw[:], qraw[:])

                    r = gu % 2           # row within ssq group (0->0, 1->32)
                    if r == 0:
                        group_state["ssq"] = pjps.tile([33, 512], dt.float32,
                                                       tag="ssq", bufs=1,
                                                       name=f"ssq{gu}")
                        group_state["members"] = []
                    ssq = group_state["ssq"]
                    group_state["members"].append((p, u, qraw))

                    def ssq_mm(ssq=ssq, r=r, q2=q2):
                        nc.tensor.matmul(ssq[32 * r:32 * r + 1, :],
                                         ones_sb[:], q2[:],
                                         start=True, stop=True)

                    gt = None
                    if r == 1:
                        mem = list(group_state["members"])

                        def group_tail(ssq=ssq, mem=mem):
                            # rows 1..31 are never written; the Ln/Exp of
                            # those lanes is garbage nothing reads.
                            lnv = work.tile([33, 512], dt.bfloat16, tag="lnv",
                                            bufs=2)
                            nc.scalar.activation(lnv[:], ssq[:], AF.Ln,
                                                 bias=eps_sb[0:33, :],
                                                 scale=1.0 / HD)
                            rs = work.tile([33, 512], dt.float32, tag="rs",
                                           bufs=2)
                            nc.scalar.activation(rs[:], lnv[:], AF.Exp,
                                                 scale=-0.5)
                            rsd = dpool.tile([2, 512], dt.float32, tag="rsd",
                                             bufs=2)
                            nc.sync.dma_start(rsd[0:1, :], rs[0:1, :])
                            nc.sync.dma_start(rsd[1:2, :], rs[32:33, :])

                            def rope_tail(mem=mem, rsd=rsd):
                                for i, (p_, u_, qraw_) in enumerate(mem):
                                    ut_, hw_ = (2, 0, 3, 1)[u_ // 4], u_ % 4
                                    tsl_ = bass.ts(hw_, 512)
                                    wnorm = qnw_sb if ut_ < 2 else knw_sb
                                    rsb = work.tile([P, 512], dt.float32,
                                                    tag="rsb", bufs=2)
                                    nc.sync.dma_start(
                                        rsb[:],
                                        rsd[i:i + 1, :].to_broadcast((P, 512)))
                                    qs = work.tile([P, 512], dt.bfloat16,
                                                   tag="qs", bufs=2)
                                    nc.vector.scalar_tensor_tensor(
                                        qs[:], qraw_[:], wnorm[:], rsb[:],
                                        op0=OP.mult, op1=OP.mult)
                                    qsw = work.tile([P, 512], dt.bfloat16,
                                                    tag="qsw", bufs=2)
                                    nc.sync.dma_start(qsw[0:64, :],
                                                      qs[64:128, :])
                                    nc.sync.dma_start(qsw[64:128, :],
                                                      qs[0:64, :])
                                    t1 = work.tile([P, 512], dt.bfloat16,
                                                   tag="t1", bufs=2)
                                    nc.vector.tensor_mul(t1[:], qs[:],
                                                         cos_sb[:, tsl_])
                                    t2 = work.tile([P, 512], dt.bfloat16,
                                                   tag="t2", bufs=2)
                                    nc.vector.tensor_mul(t2[:], qsw[:],
                                                         sin_sb[:, tsl_])
                                    nc.vector.tensor_add(
                                        qk_tiles[p_][ut_][hw_][:],
                                        t1[:], t2[:])
                            pend_rope.append(rope_tail)
                        gt = group_tail
                    proj_pend.append((gu, ssq_mm, gt))

                def attn_wave(p, l, w, o_dest):
                    """Attention for head h=2p+l, 512-query wave w."""
                    h = 2 * p + l
                    qr = qk_tiles[p][l][w]
                    kr = qk_tiles[p][2 + l]
                    qsl = bass.ts(w, 512)
                    hsl = slice(h * HD, (h + 1) * HD)
                    po = atps.tile([P, 512], dt.float32, tag="po", bufs=2)
                    es_tiles = []
                    acc = None

                    def emit_pv(kc):
                        nc.tensor.matmul(po[:], vv_sb[:, kc, hsl],
                                         es_tiles[kc // 2][:, kc % 2, :],
                                         start=(kc == 0), stop=(kc == KC - 1))

                    for kc in range(KC):
                        s2 = atps.tile([P, 512], dt.float32, tag="s2", bufs=2)
                        nc.tensor.matmul(s2[:],
                                         kr[kc // 4][:, (kc % 4) * P:
                                                     (kc % 4 + 1) * P],
                                         qr[:], start=True, stop=True)
                        if kc % 2 == 0:
                            es_tiles.append(esp.tile([P, 2, 512], dt.bfloat16,
                                                     tag="es", bufs=3,
                                                     name=f"es{h}_{w}_{kc}"))
                        nc.scalar.activation(es_tiles[-1][:, kc % 2, :], s2[:],
                                             AF.Exp, bias=expb_sb[:],
                                             scale=scale)
                        if kc == 4 and pend_dsum:
                            pend_dsum.pop(0)()
                        if kc % 2 == 1:
                            es = es_tiles[-1]
                            na = accp.tile([P, 2, 512], dt.bfloat16, tag="acc",
                                           bufs=2)
                            if acc is None:
                                nc.vector.tensor_copy(na[:], es[:])
                            else:
                                nc.vector.tensor_add(na[:], acc[:], es[:])
                            acc = na
                        if kc >= 2:
                            emit_pv(kc - 2)
                    emit_pv(KC - 2)
                    emit_pv(KC - 1)
                    dest, spill_fn = o_dest(h, qsl, w)

                    def dsum(po=po, acc=acc, dest=dest, spill_fn=spill_fn):
                        prs = atps.tile([1, 512], dt.float32, tag="prs",
                                        bufs=1)
                        for i in range(2):
                            nc.tensor.matmul(prs[:], ones_sb[:], acc[:, i, :],
                                             start=(i == 0), stop=(i == 1))
                        rr = work.tile([1, 512], dt.float32, tag="rr", bufs=2)
                        nc.vector.reciprocal_approx_fast(rr[:], prs[:])
                        rrd = dpool.tile([1, 512], dt.float32, tag="rrd",
                                         bufs=2)
                        nc.sync.dma_start(rrd[:], rr[:])
                        rrb = work.tile([P, 512], dt.float32, tag="rrb",
                                        bufs=2)
                        nc.sync.dma_start(rrb[:],
                                          rrd[:].to_broadcast((P, 512)))
                        nc.vector.tensor_mul(dest, po[:], rrb[:])
                        if spill_fn is not None:
                            spill_fn()
                    pend_dsum.append(dsum)

                # o destination during pairs 0..2: staging tile + DRAM spill
                def o_dest_spill(h, qsl, w):
                    ost = work.tile([P, 512], dt.bfloat16, tag="ost", bufs=2)

                    def spill(ost=ost, h=h, qsl=qsl):
                        nc.sync.dma_start(o_dram[h][:, qsl], ost[:])
                    return ost[:], spill

                # ---- fused steps ----
                alloc_qk(0)
                load_wqk(1)
                for u in range(NU):           # step 0: proj pair 0 only
                    proj_unit(0, u)
                    if u % 4 == 1:
                        emit_rope()
                drain_ssq(10 ** 9)
                emit_rope()

                for p in range(1, NP):        # steps 1..3
                    if p + 1 < NP:
                        load_wqk(p + 1)
                    alloc_qk(p)
                    # the last pair front-loads its projection (4 units per
                    # round) so the rms/rope tail chains drain under the
                    # remaining attention waves instead of stalling attn(3)
                    ppr = 4 if p == NP - 1 else 2
                    for j in range(8):
                        for k in range(ppr * j, min(ppr * (j + 1), NU)):
                            proj_unit(p, k)
                        l, w = divmod(j, NW)
                        attn_wave(p - 1, l, w, o_dest_spill)
                        emit_rope()
                # flush remaining dsums (spills) before the rope drain so
                # their small DMAs aren't stuck behind it on the sync queue
                for fn in pend_dsum:
                    fn()
                del pend_dsum[:]
                drain_ssq(10 ** 9)
                emit_rope()

            # x/wqk freed; load wo + reload o(h0..5) into their space
            with ExitStack() as p3x:
                wop = p3x.enter_context(tc.tile_pool(name="wop", bufs=1))
                p3op = p3x.enter_context(tc.tile_pool(name="p3op", bufs=1))
                wo_sb = wop.tile([P, NHL, DOUT], dt.bfloat16, tag="wo", bufs=1)
                wos = woT_d.rearrange("(h q) j -> q h j", q=P)
                for h in range(NHL):
                    nc.scalar.dma_start(wo_sb[:, h, :], wos[:, h, :])
                o_tiles = [p3op.tile([P, L], dt.bfloat16, tag=f"o{h}", bufs=1,
                                     name=f"o{h}") for h in range(NHL)]
                for h in range(NHL - 2):
                    nc.sync.dma_start(o_tiles[h][:], o_dram[h][:])

                # final attention: pair 3, o written directly into o_tiles
                def o_dest_direct(h, qsl, w):
                    return o_tiles[h][:, qsl], None

                for j in range(8):
                    l, w = divmod(j, NW)
                    attn_wave(NP - 1, l, w, o_dest_direct)
                for fn in pend_dsum:
                    fn()
                del pend_dsum[:]
                sbman.close()  # release the fused work pool (right side)
                man.close()    # release fused-phase PSUM pools

                # ======================= P3: wo ========================
                with ExitStack() as p3:
                    w3 = p3.enter_context(tc.tile_pool(name="w3", bufs=3))
                    p3ps = p3.enter_context(tc.tile_pool(name="p3ps", bufs=2,
                                                         space="PSUM"))
                    for tt in range(KC):
                        pw = p3ps.tile([P, JP, 512], dt.float32, tag="pw",
                                       bufs=2)
                        ttsl = bass.ts(tt, P)
                        for h in range(NHL):
                            ost = o_tiles[h][:, ttsl]
                            for jp in range(JP):
                                nc.tensor.matmul(pw[:, jp, :], ost,
                                                 wo_sb[:, h, bass.ts(jp, 512)],
                                                 start=(h == 0),
                                                 stop=(h == NHL - 1))
                        for jp in range(JP):
                            osb = w3.tile([P, 512], dt.float32, tag="outsb",
                                          bufs=3)
                            nc.vector.tensor_copy(osb[:], pw[:, jp, :])
                            eng = nc.sync if jp % 2 == 0 else nc.scalar
                            eng.dma_start(out_d[ttsl, bass.ts(jp, 512)],
                                          osb[:])

    import concourse.mybir as mybir_
    n_rm = _dedup_ldweights(nc, mybir_)
    print(f"[kernel] dedup removed {n_rm} redundant ldweights")
    nc.compile()
    return nc


def _host_prepare(x, rope_cos, rope_sin, wqkv, wo, q_norm_w, k_norm_w,
                  L, C, NP, DOUT, n_cores):
    """Build per-core input dicts."""
    NH_TOT = wqkv.shape[0] // 3 // HD
    NHL = 2 * NP
    perm = np.concatenate([np.arange(0, HD, 2), np.arange(1, HD, 2)])  # deinterleave

    qn_p = np.ascontiguousarray(q_norm_w[perm].reshape(HD, 1)).astype(np.float32)
    kn_p = np.ascontiguousarray(k_norm_w[perm].reshape(HD, 1)).astype(np.float32)

    wq = wqkv[0 * NH_TOT * HD:1 * NH_TOT * HD].reshape(NH_TOT, HD, C)
    wk = wqkv[1 * NH_TOT * HD:2 * NH_TOT * HD].reshape(NH_TOT, HD, C)
    wv = wqkv[2 * NH_TOT * HD:3 * NH_TOT * HD].reshape(NH_TOT, HD, C)

    in_maps = []
    for c in range(n_cores):
        b = c // 2
        hg = c % 2
        heads = list(range(hg * NHL, hg * NHL + NHL))
        xb = x[b * L:(b + 1) * L]                       # [L, C]
        xT = np.ascontiguousarray(xb.T).astype(BF)      # [C, L]

        qk_blocks = []
        for pidx in range(NP):
            h0, h1 = heads[2 * pidx], heads[2 * pidx + 1]
            qk_blocks += [wq[h0][perm], wq[h1][perm],
                          wk[h0][perm], wk[h1][perm]]
        wqkT = np.ascontiguousarray(
            np.concatenate(qk_blocks, axis=0).T).astype(BF)   # [C, NP*512]
        wvT = np.ascontiguousarray(
            np.concatenate([wv[h] for h in heads], axis=0).T).astype(BF)

        woT_rows = wo[:, heads[0] * HD:(heads[-1] + 1) * HD].T  # [NHL*HD, DOUT]
        woT = np.ascontiguousarray(woT_rows).astype(BF)

        cosb = rope_cos[b * L:(b + 1) * L].T            # [64, L]
        sinb = rope_sin[b * L:(b + 1) * L].T
        cosT = np.ascontiguousarray(np.concatenate([cosb, cosb], 0)).astype(BF)
        sinT = np.ascontiguousarray(np.concatenate([-sinb, sinb], 0)).astype(BF)

        in_maps.append({
            "xT": xT, "wqkT": wqkT, "wvT": wvT, "woT": woT,
            "cosT": cosT, "sinT": sinT, "qnw": qn_p, "knw": kn_p,
        })
    return in_maps


def _reference_numpy(x, rope_cos, rope_sin, cu, max_length,
                     wqkv, wo, q_norm_w, k_norm_w):
    """Pure-numpy fallback (exact reference math) for non-uniform cu."""
    T, dim = x.shape
    nh = dim // HD
    qkv = (x @ wqkv.T).reshape(T, 3, nh, HD)
    q, k, v = qkv[:, 0], qkv[:, 1], qkv[:, 2]

    def rmsnorm(t, w):
        return t / np.sqrt((t * t).mean(-1, keepdims=True) + 1e-5) * w

    def rope(t):
        tr = t.reshape(t.shape[:-1] + (HD // 2, 2))
        e, o = tr[..., 0], tr[..., 1]
        cc = rope_cos[:, None, :]
        ss = rope_sin[:, None, :]
        return np.stack([e * cc - o * ss, e * ss + o * cc], -1).reshape(t.shape)

    q = rope(rmsnorm(q, q_norm_w))
    k = rope(rmsnorm(k, k_norm_w))
    o = np.zeros((T, nh, HD), np.float32)
    nb = len(cu) - 1
    for i in range(nb):
        s, e_ = int(cu[i]), int(cu[i + 1])
        if e_ <= s:
            continue
        qs_, ks_, vs_ = q[s:e_], k[s:e_], v[s:e_]
        sc = np.einsum("lhd,mhd->hlm", qs_, ks_) / math.sqrt(HD)
        sc = sc - sc.max(-1, keepdims=True)
        a = np.exp(sc)
        a /= a.sum(-1, keepdims=True)
        o[s:e_] = np.einsum("hlm,mhd->lhd", a, vs_)
    return (o.reshape(T, dim) @ wo.T).astype(np.float32)


def kernel(x, rope_cos, rope_sin, cu, max_length, wqkv, wo, q_norm_w, k_norm_w):
    x = np.asarray(x, np.float32)
    rope_cos = np.asarray(rope_cos, np.float32)
    rope_sin = np.asarray(rope_sin, np.float32)
    cu = np.asarray(cu)
    wqkv = np.asarray(wqkv, np.float32)
    wo = np.asarray(wo, np.float32)
    q_norm_w = np.asarray(q_norm_w, np.float32)
    k_norm_w = np.asarray(k_norm_w, np.float32)

    T, C = x.shape
    N_CORES = 8
    L = T // 4
    expect_cu = np.arange(5) * L
    if (len(cu) != 5 or not np.array_equal(np.asarray(cu).ravel(), expect_cu)
            or T % 4 != 0 or L % 512 != 0 or C % P != 0):
        return _reference_numpy(x, rope_cos, rope_sin, cu, max_length,
                                wqkv, wo, q_norm_w, k_norm_w)

    NP = (C // HD) // 2 // 2          # local head pairs = NH/2/2
    DOUT = wo.shape[0]

    from concourse.bass_utils import run_bass_kernel_spmd

    nc = _build_program(L, C, NP, DOUT, N_CORES)
    in_maps = _host_prepare(x, rope_cos, rope_sin, wqkv, wo, q_norm_w, k_norm_w,
                            L, C, NP, DOUT, N_CORES)
    res = run_bass_kernel_spmd(nc, in_maps, list(range(N_CORES)))

    out = np.empty((T, DOUT), np.float32)
    for b in range(4):
        out[b * L:(b + 1) * L] = (res.results[2 * b]["out"]
                                  + res.results[2 * b + 1]["out"])
    return out
